# revision 32
# baseline (speedup 1.0000x reference)
"""GAT (3-layer, PyG-style) Trainium2 Bass kernel, 8-core SPMD — v4.

Dst-major CSR layout: nodes partitioned by dst across 8 cores (2500 each),
degree-sorted within each core so 128-dst tiles have tight max-degree k_t.
Edge slot (p, j) of a tile = j-th in-edge of the dst at tile row p, so the
chunk-j "one-hot" is the identity: segment softmax is per-partition free-axis
vector work, and scatter-add is matmul(acc, lhsT=identity, rhs=v_j) with PSUM
accumulation.

Sentinels: pad slots of REAL rows point at a -1000-score sentinel (alpha
underflows to exactly 0); entirely-padded rows (beyond nloc) point at a
ZERO sentinel (hs=0, h=0) so their Z stays finite without any +eps op.

v4: bf16 weights everywhere (LDWEIGHTS halves); BN scale folded into the
next layer's weights, BN shift applied via an extra identity-matmul into the
PSUM accumulator (epilogue = one scalar Relu, no DVE add); leaky-relu on the
scalar engine (Lrelu, hd bias fused in layer 3); the per-(j,q) alpha-multiply
is split between DVE (tensor_tensor) and the scalar engine (per-head
activation Copy with per-partition scale) to balance the two queues; all 8
epilogue transposes of a tile land in ONE PSUM bank (bf16) and are copied
out with a single op; layer-2 gather rows are bf16 768B, 8 chunks per
dma_gather; classifier + log_softmax batched over all tiles at the end.
"""

import os
import sys
from contextlib import ExitStack

import numpy as np
import ml_dtypes

for _p in ("/opt/trn_rl_repo", "/root/.axon_site/_ro/trn_rl_repo"):
    if os.path.isdir(_p) and _p not in sys.path:
        sys.path.insert(0, _p)

from concourse import bass, mybir, tile  # noqa: E402
from concourse import bacc  # noqa: E402
from concourse.bass_utils import run_bass_kernel_spmd  # noqa: E402

P = 128
DIN = 128
NCORES = 8
LEAK = 0.2
BNE = 1e-5
F32 = mybir.dt.float32
F32R = mybir.dt.float32r
BF16 = mybir.dt.bfloat16
I16 = mybir.dt.int16
AF = mybir.ActivationFunctionType
OP = mybir.AluOpType
AX = mybir.AxisListType
NPBF16 = ml_dtypes.bfloat16

ROW2 = 384   # layer-2 gather row (bf16): [h2(256) | hs2(4) | pad], 768B
ROW3 = 64    # layer-3 gather row (f32): [h3(32) | hs3(1) | pad], 256B
SENT_HS = -1000.0
GSTEP = 8    # chunks per dma_gather call
VSPLIT = 2   # every VSPLIT-th chunk's alpha-multiply runs on the scalar engine


# ---------------------------------------------------------------- host side

def _pack_idx16(flat):
    """gather position i <- idxs[i%16, i//16], replicated across the 8
    16-partition groups. flat: [m*128] -> [128, m*8]."""
    cols = len(flat) // 16
    out = np.zeros((P, cols), dtype=np.int16)
    blk = flat.reshape(cols, 16).T  # [16, cols]
    for g in range(8):
        out[16 * g:16 * (g + 1), :] = blk
    return out


def _preprocess(edge_index, n, ncores):
    nloc = n // ncores
    ntiles = (nloc + P - 1) // P
    ei = np.asarray(edge_index)
    src = np.concatenate([ei[0], np.arange(n, dtype=ei.dtype)]).astype(np.int64)
    dst = np.concatenate([ei[1], np.arange(n, dtype=ei.dtype)]).astype(np.int64)
    order = np.argsort(dst, kind="stable")
    src = src[order].astype(np.int32)
    dst = dst[order].astype(np.int32)
    deg = np.bincount(dst, minlength=n).astype(np.int64)
    starts = np.concatenate([[0], np.cumsum(deg)[:-1]])

    # degree-sort each core's local nodes (descending)
    perms, invs = [], []
    for c in range(ncores):
        dc = deg[c * nloc:(c + 1) * nloc]
        p = np.argsort(-dc, kind="stable")
        inv = np.empty(nloc, dtype=np.int64)
        inv[p] = np.arange(nloc)
        perms.append(p)
        invs.append(inv)

    # shared k per tile = max over cores
    k_list = []
    for t in range(ntiles):
        k = 1
        for c in range(ncores):
            rows = perms[c][t * P:(t + 1) * P]
            k = max(k, int(deg[c * nloc + rows].max()))
        k_list.append(k)
    offs = np.cumsum([0] + k_list)[:-1]
    nch = int(np.sum(k_list))

    # device row of original node g (for layer-2/3 gathers); each core's
    # AllGather contribution is nloc+2 rows (-1000 sentinel, zero sentinel).
    # The AllGather runs in two chunks (rows [0:cha) and [cha:nlocp)) so the
    # first chunk can fly while the last tiles still compute; the gathered
    # table is therefore chunk-major.
    nlocp = nloc + 2
    cha = min(12 * P, nloc)
    devrow = np.empty(n, dtype=np.int64)
    outrow = np.empty(n, dtype=np.int64)
    for c in range(ncores):
        inv = invs[c]
        base_b = ncores * cha + c * (nlocp - cha)
        devrow[c * nloc:(c + 1) * nloc] = np.where(
            inv < cha, c * cha + inv, base_b + (inv - cha))
        outrow[c * nloc:(c + 1) * nloc] = c * nloc + invs[c]
    sent_neg = ncores * cha + (nloc - cha)   # core-0 -1000 sentinel row
    sent_zero = sent_neg + 1

    # slot tables
    IDX1 = np.zeros((ncores, P, nch * 8), dtype=np.int16)   # ap_gather (x cols)
    IDXG = np.zeros((ncores, P, nch * 8), dtype=np.int16)   # dma_gather rows
    for c in range(ncores):
        for t in range(ntiles):
            k = k_list[t]
            off = int(offs[t])
            rows = perms[c][t * P:(t + 1) * P]  # local orig ids, len <= 128
            g = c * nloc + rows
            srccol = np.full((P, k), n, dtype=np.int64)     # -1000 x col
            srcrow = np.full((P, k), sent_neg, dtype=np.int64)
            srccol[len(g):] = n + 1                         # zero x col
            srcrow[len(g):] = sent_zero
            idx2 = starts[g][:, None] + np.arange(k)[None, :]
            m = np.arange(k)[None, :] < deg[g][:, None]
            vals = src[np.clip(idx2, 0, len(src) - 1)]
            srccol[:len(g)][m] = vals[m]
            srcrow[:len(g)][m] = devrow[vals[m]]
            # dma_gather: position i = j*128+p
            IDXG[c, :, off * 8:(off + k) * 8] = _pack_idx16(
                srcrow.T.ravel().astype(np.int16))
            # ap_gather: per chunk j, position q = column q
            for j in range(k):
                IDX1[c, :, (off + j) * 8:(off + j + 1) * 8] = _pack_idx16(
                    srccol[:, j].astype(np.int16))
    return k_list, IDX1, IDXG, perms, outrow


def _fold_weights(inp):
    out = {}
    layers = [(128, 128, 8), (1024, 64, 4), (256, 32, 1)]
    sc_prev = None
    for i, (din, c, h) in enumerate(layers, 1):
        W = np.asarray(inp[f"W{i}"], dtype=np.float64)
        a_s = np.asarray(inp[f"as{i}"], dtype=np.float64)
        a_d = np.asarray(inp[f"ad{i}"], dtype=np.float64)
        if sc_prev is not None:
            W = W * sc_prev[:, None]   # fold previous layer's BN scale
        Wr = W.reshape(din, h, c)
        Ws = np.einsum("dhc,hc->dh", Wr, a_s)
        Wd = np.einsum("dhc,hc->dh", Wr, a_d)
        out[f"Ws{i}"] = Ws
        out[f"Wd{i}"] = Wd
        out[f"W{i}"] = W
        g = np.asarray(inp[f"g{i}"], np.float64)
        be = np.asarray(inp[f"be{i}"], np.float64)
        m = np.asarray(inp[f"m{i}"], np.float64)
        v = np.asarray(inp[f"v{i}"], np.float64)
        b = np.asarray(inp[f"b{i}"], np.float64)
        sc = g / np.sqrt(v + BNE)
        assert (sc > 0).all()  # required for relu(x*sc+sh) = sc*relu(x+sh/sc)
        sh = be + (b - m) * sc
        # epilogue applies relu(acc + SH); sc is folded into the NEXT layer
        out[f"SH{i}"] = np.tile((sh / sc)[None, :], (P, 1)).astype(np.float32)
        sc_prev = sc
    # sentinel x column: quantization-aware solve so that
    # bf16(x_sent) @ bf16(Ws1) = SENT_HS for every head
    Ws1q = np.asarray(out["Ws1"].astype(NPBF16), np.float64)  # [128, 8]
    pinv = np.linalg.pinv(Ws1q.T)
    x_sent = pinv @ np.full(8, SENT_HS, np.float64)
    for _ in range(4):
        xq = np.asarray(x_sent.astype(np.float32).astype(NPBF16), np.float64)
        resid = xq @ Ws1q - SENT_HS
        x_sent = xq - pinv @ resid
    xq = np.asarray(x_sent.astype(np.float32).astype(NPBF16), np.float64)
    assert np.abs(xq @ Ws1q - SENT_HS).max() < 0.05 * abs(SENT_HS)
    assert np.abs(x_sent).max() < 1e6
    out["x_sent"] = x_sent.astype(np.float32)
    out["W1S"] = np.concatenate([out["W1"], out["Ws1"]], 1).astype(NPBF16)
    out["WD1"] = out["Wd1"].astype(NPBF16)                      # [128, 8]
    out["W2SD"] = np.concatenate(
        [out["W2"], out["Ws2"], out["Wd2"]], 1).astype(NPBF16)  # [1024, 264]
    out["W3SD"] = np.concatenate(
        [out["W3"], out["Ws3"], out["Wd3"]], 1).astype(NPBF16)  # [256, 34]
    # classifier; layer-3 BN scale folded into Wc1 rows
    wc1 = np.asarray(inp["Wc1"], np.float64) * sc_prev[:, None]
    out["WC1"] = wc1.astype(NPBF16)
    out["WC2"] = np.asarray(inp["Wc2"], np.float64).astype(NPBF16)
    out["BC1"] = np.tile(np.asarray(inp["bc1"], np.float32)[None, :], (P, 1))
    out["BC2"] = np.tile(np.asarray(inp["bc2"], np.float32)[None, :], (P, 1))
    return out


# ---------------------------------------------------------------- device side

def build_kernel(n, ncores, k_list, debug=False):
    nloc = n // ncores
    ntiles = len(k_list)
    offs = np.cumsum([0] + list(k_list))[:-1]
    nch = int(np.sum(k_list))
    kmax = max(k_list)

    nc = bacc.Bacc("TRN2", target_bir_lowering=False, debug=False,
                   num_devices=ncores)

    xt_d = nc.dram_tensor("XT", [P, n + 2], F32, kind="ExternalInput")
    xloct_d = nc.dram_tensor("XLOCT", [P, nloc], BF16, kind="ExternalInput")
    idx1_d = nc.dram_tensor("IDX1", [P, nch * 8], I16, kind="ExternalInput")
    idxg_d = nc.dram_tensor("IDXG", [P, nch * 8], I16, kind="ExternalInput")
    identb_d = nc.dram_tensor("IDENTB", [P, P], BF16, kind="ExternalInput")
    identf_d = nc.dram_tensor("IDENTF", [P, P], F32R, kind="ExternalInput")
    w1s_d = nc.dram_tensor("W1S", [P, 1032], BF16, kind="ExternalInput")
    wd1_d = nc.dram_tensor("WD1", [P, 8], BF16, kind="ExternalInput")
    w2sd_d = nc.dram_tensor("W2SD", [1024, 264], BF16, kind="ExternalInput")
    w3sd_d = nc.dram_tensor("W3SD", [256, 34], BF16, kind="ExternalInput")
    wc1_d = nc.dram_tensor("WC1", [32, 16], BF16, kind="ExternalInput")
    wc2_d = nc.dram_tensor("WC2", [16, 2], BF16, kind="ExternalInput")
    sh1_d = nc.dram_tensor("SH1", [P, 1024], F32R, kind="ExternalInput")
    sh2_d = nc.dram_tensor("SH2", [P, 256], F32R, kind="ExternalInput")
    wf_d = {}
    for nm, shp in (("SH3", [P, 32]), ("BC1", [P, 16]), ("BC2", [P, 2])):
        wf_d[nm] = nc.dram_tensor(nm, shp, F32, kind="ExternalInput")

    out_d = nc.dram_tensor("OUT", [nloc, 2], F32, kind="ExternalOutput")
    if debug:
        dbgh1_d = nc.dram_tensor("DBGH1", [P, ntiles * 8], F32,
                                 kind="ExternalOutput")
        dbg1_d = nc.dram_tensor("DBG1", [nloc, 260], BF16,
                                kind="ExternalOutput")
        dbgh2_d = nc.dram_tensor("DBGH2", [P, ntiles * 4], F32,
                                 kind="ExternalOutput")
        dbg2_d = nc.dram_tensor("DBG2", [nloc, 33], F32,
                                kind="ExternalOutput")
        dbg4_d = nc.dram_tensor("DBG4", [P, ntiles * 32], BF16,
                                kind="ExternalOutput")

    shared = "Shared" if ncores > 1 else "Local"
    nlocp = nloc + 2  # + per-core sentinel rows (-1000, zero)
    ag2_in = nc.dram_tensor("ag2_in", [nlocp, ROW2], BF16)
    ag2_out = nc.dram_tensor("ag2_out", [nlocp * ncores, ROW2], BF16,
                             addr_space=shared)
    ag3_in = nc.dram_tensor("ag3_in", [nlocp, ROW3], F32)
    ag3_out = nc.dram_tensor("ag3_out", [nlocp * ncores, ROW3], F32,
                             addr_space=shared)

    cha = min(12 * P, nloc)

    def rows_of(t):
        return min(P, nloc - t * P)

    def allgather(in_t, out_t, r0, r1):
        nc.gpsimd.collective_compute(
            "AllGather", OP.bypass,
            replica_groups=[list(range(ncores))],
            ins=[in_t[r0:r1, :]],
            outs=[out_t[ncores * r0:ncores * r1, :]])

    with ExitStack() as ctx:
        tc = ctx.enter_context(tile.TileContext(nc))
        cpool = ctx.enter_context(tc.tile_pool(name="const", bufs=1))

        identb = cpool.tile([P, P], BF16, name="identb")
        nc.sync.dma_start(out=identb[:], in_=identb_d[:, :])
        identf = cpool.tile([P, P], F32R, name="identf")
        nc.sync.dma_start(out=identf[:], in_=identf_d[:, :])
        w1s = cpool.tile([P, 1032], BF16)
        nc.sync.dma_start(out=w1s[:], in_=w1s_d[:, :])
        wd1 = cpool.tile([P, 8], BF16)
        nc.sync.dma_start(out=wd1[:], in_=wd1_d[:, :])
        w2sd = cpool.tile([P, 8, 264], BF16)
        nc.sync.dma_start(out=w2sd[:],
                          in_=w2sd_d[:, :].rearrange("(kb p) f -> p kb f", p=P))
        w3sd = cpool.tile([P, 2, 34], BF16)
        nc.sync.dma_start(out=w3sd[:],
                          in_=w3sd_d[:, :].rearrange("(kb p) f -> p kb f", p=P))
        wc1 = cpool.tile([32, 16], BF16)
        nc.sync.dma_start(out=wc1[:], in_=wc1_d[:, :])
        wc2 = cpool.tile([16, 2], BF16)
        nc.sync.dma_start(out=wc2[:], in_=wc2_d[:, :])
        sh1r = cpool.tile([P, 1024], F32R, name="sh1r")
        nc.sync.dma_start(out=sh1r[:], in_=sh1_d[:, :])
        sh2r = cpool.tile([P, 256], F32R, name="sh2r")
        nc.sync.dma_start(out=sh2r[:], in_=sh2_d[:, :])
        wf = {}
        for nm in ("SH3", "BC1", "BC2"):
            wf[nm] = cpool.tile(list(wf_d[nm].shape), F32, name=nm)
            nc.sync.dma_start(out=wf[nm][:], in_=wf_d[nm][:, :])
        idx1 = cpool.tile([P, nch * 8], I16, name="idx1")
        nc.sync.dma_start(out=idx1[:], in_=idx1_d[:, :])
        idxg = cpool.tile([P, nch * 8], I16, name="idxg")
        nc.sync.dma_start(out=idxg[:], in_=idxg_d[:, :])

        hd1_sb = cpool.tile([P, ntiles * 8], F32, name="hd1")
        nc.vector.memset(hd1_sb[:], 0.0)
        hd2_sb = cpool.tile([P, ntiles * 4], F32, name="hd2")
        hd3_sb = cpool.tile([P, ntiles], F32, name="hd3")
        x4_sb = cpool.tile([P, ntiles * 32], BF16, name="x4")
        z2_sb = cpool.tile([P, ntiles * 2], F32, name="z2")

        # per-core sentinel rows: row nloc = [0.. | hs=-1e3], row nloc+1 = 0
        sent2 = cpool.tile([2, ROW2], BF16, name="sent2")
        nc.vector.memset(sent2[:], 0.0)
        nc.vector.memset(sent2[0:1, 256:260], SENT_HS)
        nc.sync.dma_start(out=ag2_in[nloc:nloc + 2, :], in_=sent2[:])
        sent3 = cpool.tile([2, ROW3], F32, name="sent3")
        nc.vector.memset(sent3[:], 0.0)
        nc.vector.memset(sent3[0:1, 32:33], SENT_HS)
        nc.sync.dma_start(out=ag3_in[nloc:nloc + 2, :], in_=sent3[:])

        # ---- phase 0: hd1 for local nodes (perm order)
        with tc.tile_pool(name="p0", bufs=1) as pool0, \
             tc.tile_pool(name="p0p", bufs=2, space="PSUM") as pp0:
            xloct = pool0.tile([P, nloc], BF16)
            nc.sync.dma_start(out=xloct[:], in_=xloct_d[:, :])
            for t in range(ntiles):
                r = rows_of(t)
                psD = pp0.tile([P, 8], F32, tag="psD")
                nc.tensor.matmul(psD[:r], lhsT=xloct[:, t * P:t * P + r],
                                 rhs=wd1[:], start=True, stop=True)
                nc.vector.tensor_copy(out=hd1_sb[:r, t * 8:(t + 1) * 8],
                                      in_=psD[:r])

        # ================= layer 1 =================
        with tc.tile_pool(name="xtp", bufs=1) as xtp, \
             tc.tile_pool(name="L1g", bufs=2) as gpool, \
             tc.tile_pool(name="L1xg", bufs=8) as xgpool, \
             tc.tile_pool(name="L1t", bufs=1) as tailpool, \
             tc.tile_pool(name="L1s", bufs=2) as spool, \
             tc.tile_pool(name="L1v", bufs=2) as vpool, \
             tc.tile_pool(name="L1e", bufs=2) as epool, \
             tc.tile_pool(name="L1pS", bufs=1, space="PSUM") as ppS, \
             tc.tile_pool(name="L1pW", bufs=1, space="PSUM") as ppW, \
             tc.tile_pool(name="L1pH", bufs=3, space="PSUM") as ppH, \
             tc.tile_pool(name="L1pA", bufs=1, space="PSUM") as ppA:
            xt = xtp.tile([P, n + 2], F32)
            nxc = (n + 2 + 15) // 16
            for ci in range(16):
                c0, c1 = ci * nxc, min((ci + 1) * nxc, n + 2)
                nc.sync.dma_start(out=xt[:, c0:c1], in_=xt_d[:, c0:c1])

            def l1_gather(t, stash=False):
                """ap_gather + bf16 cast of tile t's x columns."""
                k = k_list[t]
                off = int(offs[t])
                if stash:
                    xgr = tailpool.tile([P, k * P], BF16, name=f"txgr{t}")
                else:
                    xgr = gpool.tile([P, kmax * P], BF16, tag="xgr")
                for j in range(k):
                    xg = xgpool.tile([P, P], F32, tag="xg")
                    nc.gpsimd.ap_gather(
                        out_ap=xg[:], in_ap=xt[:],
                        idxs_ap=idx1[:, (off + j) * 8:(off + j + 1) * 8],
                        channels=P, num_elems=n + 2, d=1, num_idxs=P)
                    nc.vector.tensor_copy(out=xgr[:, j * P:(j + 1) * P],
                                          in_=xg[:])
                return xgr

            def l1_soft(t, xgr):
                """scores + softmax for tile t; returns alpha."""
                k = k_list[t]
                off = int(offs[t])
                psS = ppS.tile([P, kmax * 8], F32, tag="psS")
                for j in range(k):
                    nc.tensor.matmul(psS[:, j * 8:(j + 1) * 8],
                                     lhsT=xgr[:, j * P:(j + 1) * P],
                                     rhs=w1s[:, 1024:1032],
                                     start=True, stop=True)
                # softmax over slots, [p, j, h] layout (j-major)
                sc = spool.tile([P, kmax * 8], F32, tag="sc")
                nc.vector.tensor_tensor(
                    out=sc[:, :8 * k].rearrange("p (j h) -> p j h", h=8),
                    in0=psS[:, :8 * k].rearrange("p (j h) -> p j h", h=8),
                    in1=hd1_sb[:, t * 8:(t + 1) * 8][:, None, :]
                        .to_broadcast([P, k, 8]),
                    op=OP.add)
                sc2 = spool.tile([P, kmax * 8], F32, tag="sc2")
                nc.vector.scalar_tensor_tensor(
                    out=sc2[:, :8 * k], in0=sc[:, :8 * k], scalar=LEAK,
                    in1=sc[:, :8 * k], op0=OP.mult, op1=OP.max)
                expsc = spool.tile([P, kmax * 8], F32, tag="expsc")
                nc.scalar.activation(out=expsc[:, :8 * k], in_=sc2[:, :8 * k],
                                     func=AF.Exp)
                z = spool.tile([P, 8], F32, tag="z")
                nc.vector.tensor_reduce(
                    out=z[:], in_=expsc[:, :8 * k].rearrange(
                        "p (j h) -> p h j", h=8),
                    axis=AX.X, op=OP.add)
                zr = spool.tile([P, 8], F32, tag="zr")
                with nc.allow_low_precision(reason="softmax 1/Z"):
                    nc.vector.reciprocal(out=zr[:], in_=z[:])
                alpha = spool.tile([P, kmax * 8], F32, tag="alpha")
                nc.vector.tensor_tensor(
                    out=alpha[:, :8 * k].rearrange("p (j h) -> p j h", h=8),
                    in0=expsc[:, :8 * k].rearrange("p (j h) -> p j h", h=8),
                    in1=zr[:, None, :].to_broadcast([P, k, 8]),
                    op=OP.mult)
                return alpha

            def l1_agg(t, xgr, alpha):
                k = k_list[t]
                r = rows_of(t)
                # weighted aggregation; BN shift seeds the accumulator
                pacc = ppA.tile([P, 1024], F32, tag="acc")
                for q in range(2):
                    qs = slice(q * 512, (q + 1) * 512)
                    nc.tensor.matmul(pacc[:, qs], lhsT=identf[:],
                                     rhs=sh1r[:, qs], start=True, stop=False)
                psh = {}

                def emit_h(j):
                    ps = []
                    for q in range(2):
                        qs = slice(q * 512, (q + 1) * 512)
                        psH = ppH.tile([P, 512], F32, tag="h")
                        nc.tensor.matmul(psH[:],
                                         lhsT=xgr[:, j * P:(j + 1) * P],
                                         rhs=w1s[:, qs], start=True, stop=True)
                        ps.append(psH)
                    psh[j] = ps

                emit_h(0)
                for j in range(k):
                    if j + 1 < k:
                        emit_h(j + 1)   # ahead of acc(j) in the tensor queue
                    v = vpool.tile([P, 1024], BF16, tag="v")
                    for q in range(2):
                        use_scalar = (q == 1)
                        psH = psh[j][q]
                        if use_scalar:
                            for hh in range(4):
                                nc.scalar.activation(
                                    out=v[:, q * 512 + hh * P:
                                          q * 512 + (hh + 1) * P],
                                    in_=psH[:, hh * P:(hh + 1) * P],
                                    func=AF.Copy,
                                    scale=alpha[:, j * 8 + q * 4 + hh:
                                                j * 8 + q * 4 + hh + 1])
                        else:
                            qs = slice(q * 512, (q + 1) * 512)
                            nc.vector.tensor_tensor(
                                out=v[:, qs].rearrange("p (h c) -> p h c", c=P),
                                in0=psH[:].rearrange("p (h c) -> p h c", c=P),
                                in1=alpha[:, j * 8 + q * 4:j * 8 + q * 4 + 4]
                                    [:, :, None].to_broadcast([P, 4, P]),
                                op=OP.mult)
                    for q in range(2):
                        qs = slice(q * 512, (q + 1) * 512)
                        nc.tensor.matmul(pacc[:, qs], lhsT=identb[:],
                                         rhs=v[:, qs],
                                         start=False, stop=(j == k - 1))
                    del psh[j]
                x2 = epool.tile([P, 1024], BF16, tag="x2")
                nc.scalar.activation(out=x2[:], in_=pacc[:], func=AF.Relu)
                # epilogue: 8 transposes -> one bf16 PSUM bank -> one copy
                tp = ppW.tile([P, 1024], BF16, tag="tp")
                for rr in range(8):
                    nc.tensor.transpose(tp[:, rr * P:(rr + 1) * P],
                                        x2[:, rr * P:(rr + 1) * P], identb[:])
                x2T = epool.tile([P, 1024], BF16, tag="x2T")
                nc.scalar.copy(out=x2T[:], in_=tp[:])
                psW = ppW.tile([P, 264], F32, tag="pw")
                for rr in range(8):
                    nc.tensor.matmul(psW[:], lhsT=x2T[:, rr * P:(rr + 1) * P],
                                     rhs=w2sd[:, rr, :],
                                     start=(rr == 0), stop=(rr == 7))
                agrow = epool.tile([P, 260], BF16, tag="agrow")
                nc.vector.tensor_copy(out=agrow[:], in_=psW[:, 0:260])
                nc.sync.dma_start(out=ag2_in[t * P:t * P + r, 0:260],
                                  in_=agrow[:r, :])
                if debug:
                    nc.sync.dma_start(out=dbg1_d[t * P:t * P + r, :],
                                      in_=agrow[:r, :])
                nc.vector.tensor_copy(out=hd2_sb[:, t * 4:(t + 1) * 4],
                                      in_=psW[:, 260:264])

            for t in range(ntiles):
                xgr = l1_gather(t)
                l1_agg(t, xgr, l1_soft(t, xgr))
                if ncores > 1 and t == cha // P - 1:
                    allgather(ag2_in, ag2_out, 0, cha)

        if debug:
            nc.sync.dma_start(out=dbgh1_d[:, :], in_=hd1_sb[:])
            nc.sync.dma_start(out=dbgh2_d[:, :], in_=hd2_sb[:])
        if ncores > 1:
            allgather(ag2_in, ag2_out, cha, nlocp)
        else:
            nc.sync.dma_start(out=ag2_out[:, :], in_=ag2_in[:, :])

        # ================= layer 2 =================
        with tc.tile_pool(name="L2g", bufs=2) as gpool, \
             tc.tile_pool(name="L2t", bufs=1) as stpool, \
             tc.tile_pool(name="L2s", bufs=2) as spool, \
             tc.tile_pool(name="L2v", bufs=2) as vpool, \
             tc.tile_pool(name="L2e", bufs=2) as epool, \
             tc.tile_pool(name="L2pS", bufs=2, space="PSUM") as ppS, \
             tc.tile_pool(name="L2pA", bufs=2, space="PSUM") as ppA:
            def l2_gather(t, stash=False):
                k = k_list[t]
                off = int(offs[t])
                if stash:
                    hg = stpool.tile([P, k * ROW2], BF16, name=f"thg{t}")
                else:
                    hg = gpool.tile([P, kmax * ROW2], BF16, tag="hg")
                for j0 in range(0, k, GSTEP):
                    kk = min(GSTEP, k - j0)
                    nc.gpsimd.dma_gather(
                        out_ap=hg[:, j0 * ROW2:(j0 + kk) * ROW2].rearrange(
                            "p (k f) -> p k f", f=ROW2),
                        in_ap=ag2_out[:, :],
                        idxs_ap=idxg[:, (off + j0) * 8:(off + j0 + kk) * 8],
                        num_idxs=kk * P, num_idxs_reg=kk * P, elem_size=ROW2)
                return hg

            def l2_comp(t, hg):
                k = k_list[t]
                off = int(offs[t])
                r = rows_of(t)
                hg3 = hg[:, :k * ROW2].rearrange("p (k f) -> p k f", f=ROW2)
                sc = spool.tile([P, kmax * 4], F32, tag="sc")
                nc.vector.tensor_tensor(
                    out=sc[:, :4 * k].rearrange("p (j h) -> p j h", h=4),
                    in0=hg3[:, :, 256:260],
                    in1=hd2_sb[:, t * 4:(t + 1) * 4][:, None, :]
                        .to_broadcast([P, k, 4]),
                    op=OP.add)
                sc2 = spool.tile([P, kmax * 4], F32, tag="sc2")
                nc.vector.scalar_tensor_tensor(
                    out=sc2[:, :4 * k], in0=sc[:, :4 * k], scalar=LEAK,
                    in1=sc[:, :4 * k], op0=OP.mult, op1=OP.max)
                expsc = spool.tile([P, kmax * 4], F32, tag="expsc")
                nc.scalar.activation(out=expsc[:, :4 * k], in_=sc2[:, :4 * k],
                                     func=AF.Exp)
                z = spool.tile([P, 4], F32, tag="z")
                nc.vector.tensor_reduce(
                    out=z[:], in_=expsc[:, :4 * k].rearrange(
                        "p (j h) -> p h j", h=4),
                    axis=AX.X, op=OP.add)
                zr = spool.tile([P, 4], F32, tag="zr")
                with nc.allow_low_precision(reason="softmax 1/Z"):
                    nc.vector.reciprocal(out=zr[:], in_=z[:])
                alpha = spool.tile([P, kmax * 4], F32, tag="alpha")
                nc.vector.tensor_tensor(
                    out=alpha[:, :4 * k].rearrange("p (j h) -> p j h", h=4),
                    in0=expsc[:, :4 * k].rearrange("p (j h) -> p j h", h=4),
                    in1=zr[:, None, :].to_broadcast([P, k, 4]),
                    op=OP.mult)
                v = vpool.tile([P, kmax * 256], BF16, tag="v")
                nc.vector.tensor_tensor(
                    out=v[:, :k * 256].rearrange(
                        "p (j h c) -> p j h c", h=4, c=64),
                    in0=hg3[:, :, 0:256].rearrange(
                        "p j (h c) -> p j h c", c=64),
                    in1=alpha[:, :4 * k].rearrange(
                        "p (j h) -> p j h", h=4)[:, :, :, None]
                        .to_broadcast([P, k, 4, 64]),
                    op=OP.mult)
                pacc = ppA.tile([P, 256], F32, tag="acc")
                nc.tensor.matmul(pacc[:], lhsT=identf[:], rhs=sh2r[:],
                                 start=True, stop=False)
                for j in range(k):
                    nc.tensor.matmul(pacc[:], lhsT=identb[:],
                                     rhs=v[:, j * 256:(j + 1) * 256],
                                     start=False, stop=(j == k - 1))
                x3 = epool.tile([P, 256], BF16, tag="x3")
                nc.scalar.activation(out=x3[:], in_=pacc[:], func=AF.Relu)
                tp = ppS.tile([P, 256], BF16, tag="tp")
                for rr in range(2):
                    nc.tensor.transpose(tp[:, rr * P:(rr + 1) * P],
                                        x3[:, rr * P:(rr + 1) * P], identb[:])
                x3T = epool.tile([P, 256], BF16, tag="x3T")
                nc.vector.tensor_copy(out=x3T[:], in_=tp[:])
                psW = ppA.tile([P, 34], F32, tag="pw")
                for rr in range(2):
                    nc.tensor.matmul(psW[:], lhsT=x3T[:, rr * P:(rr + 1) * P],
                                     rhs=w3sd[:, rr, :],
                                     start=(rr == 0), stop=(rr == 1))
                agrow = epool.tile([P, 33], F32, tag="agrow")
                nc.vector.tensor_copy(out=agrow[:], in_=psW[:, 0:33])
                nc.sync.dma_start(out=ag3_in[t * P:t * P + r, 0:33],
                                  in_=agrow[:r, :])
                if debug:
                    nc.sync.dma_start(out=dbg2_d[t * P:t * P + r, :],
                                      in_=agrow[:r, :])
                nc.vector.tensor_copy(out=hd3_sb[:, t:t + 1],
                                      in_=psW[:, 33:34])

            for t in range(ntiles):
                l2_comp(t, l2_gather(t))
                if ncores > 1 and t == cha // P - 1:
                    allgather(ag3_in, ag3_out, 0, cha)

        if ncores > 1:
            allgather(ag3_in, ag3_out, cha, nlocp)
        else:
            nc.sync.dma_start(out=ag3_out[:, :], in_=ag3_in[:, :])

        # ================= layer 3 (+ batched classifier) =================
        with tc.tile_pool(name="L3g", bufs=2) as gpool, \
             tc.tile_pool(name="L3s", bufs=2) as spool, \
             tc.tile_pool(name="L3c", bufs=1) as clpool, \
             tc.tile_pool(name="L3e", bufs=2) as epool, \
             tc.tile_pool(name="L3p", bufs=2, space="PSUM") as pp3:
            for t in range(ntiles):
                k = k_list[t]
                off = int(offs[t])
                hg = gpool.tile([P, kmax * ROW3], F32, tag="hg")
                for j0 in range(0, k, GSTEP):
                    kk = min(GSTEP, k - j0)
                    nc.gpsimd.dma_gather(
                        out_ap=hg[:, j0 * ROW3:(j0 + kk) * ROW3].rearrange(
                            "p (k f) -> p k f", f=ROW3),
                        in_ap=ag3_out[:, :],
                        idxs_ap=idxg[:, (off + j0) * 8:(off + j0 + kk) * 8],
                        num_idxs=kk * P, num_idxs_reg=kk * P, elem_size=ROW3)
                hg3 = hg[:, :k * ROW3].rearrange("p (k f) -> p k f", f=ROW3)
                sc = spool.tile([P, kmax], F32, tag="sc")
                nc.vector.tensor_tensor(
                    out=sc[:, :k], in0=hg3[:, :, 32],
                    in1=hd3_sb[:, t:t + 1].to_broadcast([P, k]), op=OP.add)
                sc2 = spool.tile([P, kmax], F32, tag="sc2")
                nc.vector.scalar_tensor_tensor(
                    out=sc2[:, :k], in0=sc[:, :k], scalar=LEAK,
                    in1=sc[:, :k], op0=OP.mult, op1=OP.max)
                expsc = spool.tile([P, kmax], F32, tag="expsc")
                z = spool.tile([P, 1], F32, tag="z")
                nc.scalar.activation(out=expsc[:, :k], in_=sc2[:, :k],
                                     func=AF.Exp, accum_out=z[:])
                zr = spool.tile([P, 1], F32, tag="zr")
                with nc.allow_low_precision(reason="softmax 1/Z"):
                    nc.vector.reciprocal(out=zr[:], in_=z[:])
                alpha = spool.tile([P, kmax], F32, tag="alpha")
                nc.vector.tensor_tensor(out=alpha[:, :k], in0=expsc[:, :k],
                                        in1=zr[:].to_broadcast([P, k]),
                                        op=OP.mult)
                v = spool.tile([P, kmax * 32], F32, tag="v")
                nc.vector.tensor_tensor(
                    out=v[:, :k * 32].rearrange("p (j c) -> p j c", c=32),
                    in0=hg3[:, :, 0:32],
                    in1=alpha[:, :k, None].to_broadcast([P, k, 32]),
                    op=OP.mult)
                acc3 = spool.tile([P, 32], F32, tag="acc3")
                nc.vector.tensor_reduce(
                    out=acc3[:], in_=v[:, :k * 32].rearrange(
                        "p (j c) -> p c j", c=32),
                    axis=AX.X, op=OP.add)
                x4f = spool.tile([P, 32], F32, tag="x4f")
                nc.vector.tensor_tensor(out=x4f[:], in0=acc3[:],
                                        in1=wf["SH3"][:], op=OP.add)
                nc.scalar.activation(out=x4_sb[:, t * 32:(t + 1) * 32],
                                     in_=x4f[:], func=AF.Relu)
            if debug:
                nc.sync.dma_start(out=dbg4_d[:, :], in_=x4_sb[:])
            # ---- classifier over all tiles (phase-batched)
            x4T = clpool.tile([32, ntiles * P], BF16, name="x4T")
            z1T = clpool.tile([16, ntiles * P], BF16, name="z1T")
            z1b = clpool.tile([P, ntiles * 16], BF16, name="z1b")
            TB = 8  # transposes batched into one PSUM bank
            for t0 in range(0, ntiles, TB):
                tb = min(TB, ntiles - t0)
                tpx = pp3.tile([32, TB * P], BF16, tag="tpx")
                for i in range(tb):
                    nc.tensor.transpose(
                        tpx[:, i * P:(i + 1) * P],
                        x4_sb[:, (t0 + i) * 32:(t0 + i + 1) * 32], identb[:])
                nc.vector.tensor_copy(out=x4T[:, t0 * P:(t0 + tb) * P],
                                      in_=tpx[:, 0:tb * P])
            for t in range(ntiles):
                z1p = pp3.tile([P, 16], F32, tag="z1p")
                nc.tensor.matmul(z1p[:], lhsT=x4T[:, t * P:(t + 1) * P],
                                 rhs=wc1[:], start=True, stop=True)
                z1f = epool.tile([P, 16], F32, tag="z1f")
                nc.vector.tensor_tensor(out=z1f[:], in0=z1p[:],
                                        in1=wf["BC1"][:], op=OP.add)
                nc.scalar.activation(out=z1b[:, t * 16:(t + 1) * 16],
                                     in_=z1f[:], func=AF.Relu)
            for t0 in range(0, ntiles, TB):
                tb = min(TB, ntiles - t0)
                tpz = pp3.tile([16, TB * P], BF16, tag="tpz")
                for i in range(tb):
                    nc.tensor.transpose(
                        tpz[:, i * P:(i + 1) * P],
                        z1b[:, (t0 + i) * 16:(t0 + i + 1) * 16], identb[:])
                nc.vector.tensor_copy(out=z1T[:, t0 * P:(t0 + tb) * P],
                                      in_=tpz[:, 0:tb * P])
            for t in range(ntiles):
                z2p = pp3.tile([P, 2], F32, tag="z2p")
                nc.tensor.matmul(z2p[:], lhsT=z1T[:, t * P:(t + 1) * P],
                                 rhs=wc2[:], start=True, stop=True)
                nc.vector.tensor_tensor(out=z2_sb[:, t * 2:(t + 1) * 2],
                                        in0=z2p[:], in1=wf["BC2"][:],
                                        op=OP.add)
            # ---- log_softmax over all tiles at once (2 table loads total)
            ez = clpool.tile([P, ntiles * 2], F32, name="ez")
            nc.scalar.activation(out=ez[:], in_=z2_sb[:], func=AF.Exp)
            zs = clpool.tile([P, ntiles], F32, name="zs")
            ez3 = ez[:].rearrange("p (t c) -> p t c", c=2)
            nc.vector.tensor_tensor(out=zs[:], in0=ez3[:, :, 0],
                                    in1=ez3[:, :, 1], op=OP.add)
            lse = clpool.tile([P, ntiles], F32, name="lse")
            nc.scalar.activation(out=lse[:], in_=zs[:], func=AF.Ln)
            res = clpool.tile([P, ntiles * 2], F32, name="res")
            nc.vector.tensor_tensor(
                out=res[:].rearrange("p (t c) -> p t c", c=2),
                in0=z2_sb[:].rearrange("p (t c) -> p t c", c=2),
                in1=lse[:, :, None].to_broadcast([P, ntiles, 2]),
                op=OP.subtract)
            full = nloc // P
            rem = nloc - full * P
            nc.sync.dma_start(
                out=out_d[0:full * P, :].rearrange("(t p) f -> p t f", p=P),
                in_=res[:, 0:full * 2].rearrange("p (t c) -> p t c", c=2))
            if rem:
                nc.sync.dma_start(
                    out=out_d[full * P:nloc, :],
                    in_=res[0:rem, full * 2:full * 2 + 2])

    nc.compile()
    return nc


# ---------------------------------------------------------------- entry point

_CACHE = {}


def make_in_maps(inputs, ncores=NCORES):
    edge_index = np.asarray(inputs["edge_index"])
    x = np.asarray(inputs["x"], dtype=np.float32)
    n = x.shape[0]
    nloc = n // ncores
    k_list, IDX1, IDXG, perms, outrow = _preprocess(edge_index, n, ncores)
    w = _fold_weights(inputs)

    # [128, n+2]: x columns | -1000-sentinel | zero-sentinel
    xt = np.concatenate(
        [x.T, w["x_sent"][:, None], np.zeros((P, 1), np.float32)], axis=1)
    identb = np.ascontiguousarray(np.eye(P, dtype=NPBF16))
    identf = np.ascontiguousarray(np.eye(P, dtype=np.float32))
    base = dict(XT=np.ascontiguousarray(xt), IDENTB=identb, IDENTF=identf)
    for nm in ("W1S", "WD1", "W2SD", "W3SD", "WC1", "WC2", "BC1", "BC2"):
        base[nm] = np.ascontiguousarray(w[nm])
    base["SH1"] = np.ascontiguousarray(w["SH1"])
    base["SH2"] = np.ascontiguousarray(w["SH2"])
    base["SH3"] = np.ascontiguousarray(w["SH3"])
    in_maps = []
    for c in range(ncores):
        m = dict(base)
        loc = x[c * nloc + perms[c]]
        m["XLOCT"] = np.ascontiguousarray(loc.T.astype(NPBF16))
        m["IDX1"] = np.ascontiguousarray(IDX1[c])
        m["IDXG"] = np.ascontiguousarray(IDXG[c])
        in_maps.append(m)
    return n, k_list, in_maps, outrow


def kernel(**inputs):
    n, k_list, in_maps, outrow = make_in_maps(inputs)
    key = (n, tuple(k_list))
    if key not in _CACHE:
        _CACHE[key] = build_kernel(n, NCORES, k_list)
    nc = _CACHE[key]
    res = run_bass_kernel_spmd(nc, in_maps, core_ids=list(range(NCORES)))
    allout = np.concatenate([r["OUT"] for r in res.results], axis=0)
    return allout[outrow].astype(np.float32)
